# revision 22
# baseline (speedup 1.0000x reference)
"""BitLinear (int8-activation x ternary-weight) matmul on 8 TRN2 NeuronCores.

Full inputs: x [4, 4096, 2048] f32, weight [2048, 2048] f32.
Output: [4, 4096, 2048] fp16 = ((qx @ qw.T) / si / sw).astype(f16).

Strategy: data-parallel over the 16384 rows (2048 rows/core). W streams
once on the sync DMA queue; mean|W| accumulates tile-by-tile on ACT
(Abs + accum_out) as tiles land, so sw is ready ~2us after the last
tile. 12 raw tiles stay cached in SBUF; the first 4 are re-read right
behind the stream and quantized last. Ternary quantize (DVE magic-round
then Sign->fp8) is split across ACT and DVE so the serial tail after sw
is ~26us; row tiles 0-1 interleave their matmuls right behind it (PSUM
fits exactly 2 row tiles). PE idle-clock (HAM) stays warm via tiny
fp32 matmuls paced by the W-tile arrivals - a dense junk flood trips
the power throttle instead. Activations quantize per-row to int8 via
DVE (scale+magic-round), bf16 qx, DMA-xbar transpose on the sync queue.

Matmul: KTE "exact" k-planes run bf16(qx^T) x fp8(qw^T) with fp32 PSUM
accumulation (bit-exact int8 x ternary); the remaining planes pair up
(plane, plane+1) as fp8 DoubleRow instructions with qx RNE-rounded to
fp8 (lossy; rel-err ~1.7e-2 at KTE=10, within the 2e-2 gate), which
processes 2 k-planes per 512-cycle instruction - 157 TF/s. Dequant is fused into the
PSUM->SBUF fp16 copy on ACT. Host only reshapes/shards and transposes
W (layout prep, no math).
"""

import numpy as np

import concourse.mybir as mybir
import concourse.tile as tile
from concourse import bacc
from concourse.bass import ts
from concourse.bass_utils import run_bass_kernel_spmd

N_CORES = 8
ROWS_TOTAL = 4 * 4096
K = 2048
N = 2048
MAGIC = 12582912.0  # 1.5*2^23: fp32 round-to-nearest-even (both signs)
NPRE = 3  # x tiles prefetched/quantized before the ramp
NCACHE = 12  # wld pool bufs: W tiles resident when sw becomes known
NDVEQ = 3  # trailing quantize slots handled fully on DVE (rest: ACT Sign)
KTE = 10  # exact k-planes (hi/lo split); KT-KTE lossy fp8 planes (even)

f32 = mybir.dt.float32
bf16 = mybir.dt.bfloat16
f16 = mybir.dt.float16
fp8 = mybir.dt.float8e4
Alu = mybir.AluOpType
Act = mybir.ActivationFunctionType
AxX = mybir.AxisListType.X
DR = mybir.MatmulPerfMode.DoubleRow


def build(rows_per_core=ROWS_TOTAL // N_CORES):
    nc = bacc.Bacc(
        "TRN2", target_bir_lowering=False, debug=False, num_devices=N_CORES
    )
    x_ext = nc.declare_dram_parameter("x", [rows_per_core, K], f32, isOutput=False)
    wt_ext = nc.declare_dram_parameter("wt", [K, N], f32, isOutput=False)
    out_ext = nc.declare_dram_parameter(
        "out", [rows_per_core, N], f16, isOutput=True
    )

    KT = K // 128
    MT = rows_per_core // 128
    NQ = N // 512
    KTL = KT - KTE  # lossy planes: [KTE, KT), paired (KTE,KTE+1),...
    assert KTL % 2 == 0
    NRR = KT - NCACHE  # re-read tiles (quantized last)
    # quantize order: resident tiles first, re-read tiles last
    qorder = list(range(NRR, KT)) + list(range(NRR))
    # matmul step list per row tile: exact planes singly, lossy planes as
    # pairs once both are quantized; same order as quantize availability
    seen = set()
    steps = []  # (kind, kt)
    for kt in qorder:
        seen.add(kt)
        if kt < KTE:
            steps.append(("e", kt))
        else:
            mate = kt + 1 if kt % 2 == KTE % 2 else kt - 1
            if mate in seen:
                steps.append(("l", min(kt, mate)))

    with tile.TileContext(nc) as tc:
        with (
            tc.tile_pool(name="xin", bufs=4) as xin,  # [128,K] f32 x loads
            tc.tile_pool(name="wld", bufs=NCACHE) as wld,  # [128,K] f32 W
            tc.tile_pool(name="qtmp", bufs=2) as qtmp,  # qx bf16
            tc.tile_pool(name="qxt", bufs=4) as qxtp,  # [128,KT,128] bf16 x^T
            tc.tile_pool(name="qxl", bufs=4) as qxlp,  # [128,KTL,128] fp8
            tc.tile_pool(name="outp", bufs=2) as outp,  # [128,N] f16 results
            tc.tile_pool(name="scr", bufs=1) as scr,  # [128,K] bf16 |w| out
            tc.tile_pool(name="singles", bufs=1) as singles,
            tc.tile_pool(name="small", bufs=6) as small,  # [128,1] stats
            tc.tile_pool(name="pacc", bufs=8, space="PSUM") as pacc,
        ):
            ones_mat = singles.tile([128, 128], f32)
            nc.vector.memset(ones_mat, 1.0)
            qwT = singles.tile([128, KT, N], fp8)
            wsums = singles.tile([128, KT], f32)
            negmagic = singles.tile([128, 1], f32)
            nc.vector.memset(negmagic, -MAGIC)

            # ---- W stream on sync with x0..x2 loads slotted mid-stream
            # (late enough not to delay the mean, early enough for the ramp);
            # the scalar queue carries only ACT work so Abs starts immediately
            x_pre = {}

            def x_load(mi):
                x_t = xin.tile([128, K], f32, tag="xin", name=f"xl{mi}")
                nc.sync.dma_start(out=x_t, in_=x_ext[ts(mi, 128), :])
                x_pre[mi] = x_t

            # W stream split across both HW DMA queues (evens on sync with
            # the x prefetches, odds on scalar) so the mean pass is ACT-
            # bound (~40us) instead of single-queue DMA-bound (~66us).
            # wld13/wld15 reuse the slots of wld1/wld3, whose release needs
            # Abs1/Abs3 - those two descriptors are deferred into the Abs
            # series below so they never head-of-line block the ACT queue.
            wstream = {}
            for kt in range(KT):
                wstream[kt] = wld.tile(
                    [128, K], f32, tag="wld", name=f"wld{kt}"
                )
            for kt in range(1, KT - NCACHE + 8, 2):
                nc.scalar.dma_start(
                    out=wstream[kt], in_=wt_ext[ts(kt, 128), :]
                )
            nev = 0
            for kt in range(0, KT, 2):
                nc.sync.dma_start(
                    out=wstream[kt], in_=wt_ext[ts(kt, 128), :]
                )
                nev += 1
                if 6 <= nev < 6 + min(NPRE, MT):
                    x_load(nev - 6)
            for kt in range(KT):
                wt_t = wstream[kt]
                aw = scr.tile([128, K], bf16, tag="scr")
                nc.scalar.activation(
                    out=aw, in_=wt_t, func=Act.Abs,
                    accum_out=wsums[:, kt : kt + 1],
                )
                if kt in (1, 3) and kt + 12 < KT:
                    nc.scalar.dma_start(
                        out=wstream[kt + 12], in_=wt_ext[ts(kt + 12, 128), :]
                    )
                for j in range(2):
                    pj = pacc.tile(
                        [128, 64], f32, tag="acc", name=f"jk{kt}_{j}"
                    )
                    nc.tensor.matmul(
                        pj, lhsT=wt_t[:, :128],
                        rhs=wt_t[:, 128 + 64 * j : 192 + 64 * j],
                        start=True, stop=True, skip_group_check=True,
                    )

            def x_quant(mi):
                if mi in x_pre:
                    x_t = x_pre[mi]
                else:
                    x_t = xin.tile([128, K], f32, tag="xin", name=f"x{mi}")
                    nc.sync.dma_start(out=x_t, in_=x_ext[ts(mi, 128), :])
                amax = small.tile([128, 1], f32, tag="small")
                nc.vector.tensor_reduce(
                    out=amax, in_=x_t, axis=AxX, op=Alu.max,
                    apply_absolute_value=True,
                )
                amc = small.tile([128, 1], f32, tag="amc", name=f"amc{mi}")
                nc.vector.tensor_scalar_max(out=amc, in0=amax, scalar1=1e-5)
                rec = small.tile([128, 1], f32, tag="small")
                nc.vector.reciprocal(out=rec, in_=amc)
                si = small.tile([128, 1], f32, tag="small")
                nc.vector.tensor_scalar_mul(out=si, in0=rec, scalar1=127.0)
                # u = x*si + MAGIC in place (DVE), qx = u - MAGIC -> bf16
                nc.vector.tensor_scalar(
                    out=x_t, in0=x_t, scalar1=si, scalar2=MAGIC,
                    op0=Alu.mult, op1=Alu.add,
                )
                qx = qtmp.tile([128, K], bf16, tag="qtmp")
                nc.vector.tensor_scalar_add(out=qx, in0=x_t, scalar1=-MAGIC)
                qxT = qxtp.tile(
                    [128, KT, 128], bf16, tag="qxt", name=f"qxT{mi}"
                )
                nc.sync.dma_start_transpose(out=qxT, in_=qx)
                # RNE fp8 copies of the lossy planes (paired for DoubleRow)
                qxl = qxlp.tile(
                    [128, KTL, 128], fp8, tag="qxl", name=f"qxl{mi}"
                )
                nc.vector.tensor_copy(out=qxl, in_=qxT[:, KTE:, :])
                return qxT, qxl, amc

            xq = {}
            for mi in range(min(NPRE, MT)):
                xq[mi] = x_quant(mi)

            wtot = small.tile([128, 1], f32, tag="small")
            nc.vector.tensor_reduce(out=wtot, in_=wsums, axis=AxX, op=Alu.add)

            # ---- mean broadcast + scalar chain (before the re-read junk on
            # the PE queue: the re-reads wait on slot releases that need sw)
            ptot_b = pacc.tile([128, 1], f32, tag="acc", name="ptot_b")
            nc.tensor.matmul(ptot_b, lhsT=ones_mat, rhs=wtot, start=True, stop=True)

            # re-read of the first NRR tiles, right behind the stream.
            # No junk matmuls on them: they would head-of-line block the
            # ramp matmuls behind the slot-WAR'd re-read DMAs. The clock
            # bridge to the first real matmul is 3 tiny matmuls on the
            # freshly transposed qxT tiles instead.
            for kt in range(NRR):
                wt_t = wld.tile([128, K], f32, tag="wld", name=f"wldr{kt}")
                nc.sync.dma_start(out=wt_t, in_=wt_ext[ts(kt, 128), :])
                wstream[kt] = wt_t
            for mi in range(min(NPRE, MT)):
                qxT_j = xq[mi][0]
                pj = pacc.tile([128, 128], f32, tag="acc", name=f"jt{mi}")
                nc.tensor.matmul(
                    pj, lhsT=qxT_j[:, 0, :], rhs=qxT_j[:, 1, :],
                    start=True, stop=True, skip_group_check=True,
                )
            meanc_b = small.tile([128, 1], f32, tag="s1")
            nc.vector.tensor_scalar(
                out=meanc_b,
                in0=ptot_b,
                scalar1=1.0 / (K * N),
                scalar2=1e-5,
                op0=Alu.mult,
                op1=Alu.max,
            )
            sw_b = singles.tile([128, 1], f32)
            nc.vector.reciprocal(out=sw_b, in_=meanc_b)
            q_b = singles.tile([128, 1], f32)
            nc.vector.tensor_scalar_mul(out=q_b, in0=meanc_b, scalar1=1.0 / 127.0)

            # ---- W quantize, split across engines: DVE magic-round, then
            # ACT Sign(u-MAGIC)->fp8 for most planes; the last NDVEQ slots
            # run fully on DVE (sub+max, then min->fp8)
            def w_quant(qi, kt):
                wt_t = wstream[kt]
                nc.vector.tensor_scalar(
                    out=wt_t, in0=wt_t, scalar1=sw_b, scalar2=MAGIC,
                    op0=Alu.mult, op1=Alu.add,
                )
                if qi < KT - NDVEQ:
                    nc.scalar.activation(
                        out=qwT[:, kt, :], in_=wt_t, func=Act.Sign,
                        bias=negmagic,
                    )
                else:
                    nc.vector.tensor_scalar(
                        out=wt_t, in0=wt_t, scalar1=-MAGIC, scalar2=-1.0,
                        op0=Alu.add, op1=Alu.max,
                    )
                    nc.vector.tensor_scalar(
                        out=qwT[:, kt, :], in0=wt_t, scalar1=1.0, scalar2=1.0,
                        op0=Alu.mult, op1=Alu.min,
                    )

            # ---- matmuls: bf16 x fp8 for exact planes; fp8 DoubleRow
            # pairs (2 k-planes per instr) for the lossy planes
            def mm(acc, qxT, qxl, step, nq, start, stop):
                kind, kt = step
                if kind == "e":
                    nc.tensor.matmul(
                        acc, lhsT=qxT[:, kt, :], rhs=qwT[:, kt, ts(nq, 512)],
                        start=start, stop=stop, skip_group_check=True,
                    )
                else:
                    nc.tensor.matmul(
                        acc,
                        lhsT=qxl[:, kt - KTE : kt - KTE + 2, :],
                        rhs=qwT[:, kt : kt + 2, ts(nq, 512)],
                        start=start, stop=stop,
                        perf_mode=DR, skip_group_check=True,
                    )

            def finish(mi, accs, amc, split=False):
                cs = small.tile([128, 1], f32, tag="small")
                nc.vector.tensor_mul(cs, amc, q_b)  # (amax/127)*meanc
                o_t = outp.tile([128, N], f16, tag="outp", name=f"o{mi}")
                for nq in range(NQ):
                    nc.scalar.activation(
                        out=o_t[:, ts(nq, 512)], in_=accs[nq],
                        func=Act.Copy, scale=cs,
                    )
                    if split:
                        nc.scalar.dma_start(
                            out=out_ext[ts(mi, 128), ts(nq, 512)],
                            in_=o_t[:, ts(nq, 512)],
                        )
                if not split:
                    nc.scalar.dma_start(out=out_ext[ts(mi, 128), :], in_=o_t)

            NS = len(steps)
            if MT >= 2:
                # ramp: row tiles 0,1 interleaved across the quantize burst
                qxT0, qxl0, amc0 = xq[0]
                qxT1, qxl1, amc1 = xq[1]
                accs0 = [
                    pacc.tile([128, 512], f32, tag="acc", name=f"acc_0_{i}")
                    for i in range(NQ)
                ]
                accs1 = [
                    pacc.tile([128, 512], f32, tag="acc", name=f"acc_1_{i}")
                    for i in range(NQ)
                ]
                qdone = 0
                for si_, step in enumerate(steps):
                    # emit quantizes needed for this step
                    need = qorder.index(step[1]) + 1 if step[0] == "e" else max(
                        qorder.index(step[1]), qorder.index(step[1] + 1)
                    ) + 1
                    while qdone < need:
                        w_quant(qdone, qorder[qdone])
                        qdone += 1
                    st, sp = si_ == 0, si_ == NS - 1
                    for nq in range(NQ):
                        mm(accs0[nq], qxT0, qxl0, step, nq, st, sp)
                    for nq in range(NQ):
                        mm(accs1[nq], qxT1, qxl1, step, nq, st, sp)
                while qdone < KT:
                    w_quant(qdone, qorder[qdone])
                    qdone += 1
                finish(0, accs0, amc0)
                finish(1, accs1, amc1)
                start_mi = 2
                for la in range(NPRE, min(NPRE + 2, MT)):
                    x_load(la)
            else:
                for qi, kt in enumerate(qorder):
                    w_quant(qi, kt)
                start_mi = 0

            for mi in range(start_mi, MT):
                qxT, qxl, amc = xq[mi] if mi in xq else x_quant(mi)
                if mi + 2 + NPRE - start_mi < MT and mi >= NPRE - 1:
                    x_load(mi + 2 + NPRE - start_mi)
                accs = [
                    pacc.tile([128, 512], f32, tag="acc", name=f"acc_{mi}_{i}")
                    for i in range(NQ)
                ]
                if mi == MT - 1:
                    # nq-inner: shorter kernel tail
                    for nq in range(NQ):
                        for si_, step in enumerate(steps):
                            mm(accs[nq], qxT, qxl, step, nq,
                               si_ == 0, si_ == NS - 1)
                else:
                    for si_, step in enumerate(steps):
                        st, sp = si_ == 0, si_ == NS - 1
                        for nq in range(NQ):
                            mm(accs[nq], qxT, qxl, step, nq, st, sp)
                finish(mi, accs, amc, split=(mi == MT - 1))

    nc.compile()
    return nc


_NC_CACHE = {}


def _get_nc(rows_per_core):
    if rows_per_core not in _NC_CACHE:
        _NC_CACHE[rows_per_core] = build(rows_per_core)
    return _NC_CACHE[rows_per_core]


def run(x, weight, **spmd_kwargs):
    x = np.ascontiguousarray(np.asarray(x, dtype=np.float32))
    weight = np.asarray(weight, dtype=np.float32)
    b, s, k = x.shape
    rows = b * s
    rpc = rows // N_CORES
    xr = x.reshape(rows, k)
    wt = np.ascontiguousarray(weight.T)
    nc = _get_nc(rpc)
    in_maps = [
        {"x": xr[i * rpc : (i + 1) * rpc], "wt": wt} for i in range(N_CORES)
    ]
    res = run_bass_kernel_spmd(
        nc, in_maps, core_ids=list(range(N_CORES)), **spmd_kwargs
    )
    out = np.concatenate(
        [res.results[i]["out"] for i in range(N_CORES)], axis=0
    )
    return out.reshape(b, s, N), res


def kernel(x, weight):
    out, _ = run(x, weight)
    return out


# revision 24
# speedup vs baseline: 1.2334x; 1.2334x over previous
"""BitLinear (int8-activation x ternary-weight) matmul on 8 TRN2 NeuronCores.

Full inputs: x [4, 4096, 2048] f32, weight [2048, 2048] f32.
Output: [4, 4096, 2048] fp16 = ((qx @ qw.T) / si / sw).astype(f16).

Strategy: data-parallel over the 16384 rows (2048 rows/core). W streams
once on the sync DMA queue; mean|W| accumulates tile-by-tile on ACT
(Abs + accum_out) as tiles land, so sw is ready ~2us after the last
tile. 12 raw tiles stay cached in SBUF; the first 4 are re-read right
behind the stream and quantized last. Ternary quantize (DVE magic-round
then Sign->fp8) is split across ACT and DVE so the serial tail after sw
is ~26us; row tiles 0-1 interleave their matmuls right behind it (PSUM
fits exactly 2 row tiles). PE idle-clock (HAM) stays warm via tiny
fp32 matmuls paced by the W-tile arrivals - a dense junk flood trips
the power throttle instead. Activations quantize per-row to int8 via
DVE (scale+magic-round), bf16 qx, DMA-xbar transpose on the sync queue.

Matmul: KTE "exact" k-planes run bf16(qx^T) x fp8(qw^T) with fp32 PSUM
accumulation (bit-exact int8 x ternary); the remaining planes pair up
(plane, plane+1) as fp8 DoubleRow instructions with qx RNE-rounded to
fp8 (lossy; rel-err ~1.7e-2 at KTE=10, within the 2e-2 gate), which
processes 2 k-planes per 512-cycle instruction - 157 TF/s. Dequant is fused into the
PSUM->SBUF fp16 copy on ACT. Host only reshapes/shards and transposes
W (layout prep, no math).
"""

import numpy as np

import concourse.mybir as mybir
import concourse.tile as tile
from concourse import bacc
from concourse.bass import ts
from concourse.bass_utils import run_bass_kernel_spmd

N_CORES = 8
ROWS_TOTAL = 4 * 4096
K = 2048
N = 2048
MAGIC = 12582912.0  # 1.5*2^23: fp32 round-to-nearest-even (both signs)
NPRE = 3  # x tiles prefetched/quantized before the ramp
NCACHE = 12  # wld pool bufs: W tiles resident when sw becomes known
NDVEQ = 3  # trailing quantize slots handled fully on DVE (rest: ACT Sign)
KTE = 10  # exact k-planes (hi/lo split); KT-KTE lossy fp8 planes (even)

f32 = mybir.dt.float32
bf16 = mybir.dt.bfloat16
f16 = mybir.dt.float16
fp8 = mybir.dt.float8e4
Alu = mybir.AluOpType
Act = mybir.ActivationFunctionType
AxX = mybir.AxisListType.X
DR = mybir.MatmulPerfMode.DoubleRow


def build(rows_per_core=ROWS_TOTAL // N_CORES):
    nc = bacc.Bacc(
        "TRN2", target_bir_lowering=False, debug=False, num_devices=N_CORES
    )
    x_ext = nc.declare_dram_parameter("x", [rows_per_core, K], f32, isOutput=False)
    wt_ext = nc.declare_dram_parameter("wt", [K, N], f32, isOutput=False)
    out_ext = nc.declare_dram_parameter(
        "out", [rows_per_core, N], f16, isOutput=True
    )

    KT = K // 128
    MT = rows_per_core // 128
    NQ = N // 512
    KTL = KT - KTE  # lossy planes: [KTE, KT), paired (KTE,KTE+1),...
    assert KTL % 2 == 0
    NRR = KT - NCACHE  # re-read tiles (quantized last)
    # quantize order: resident tiles first, re-read tiles last
    qorder = list(range(NRR, KT)) + list(range(NRR))
    # matmul step list per row tile: exact planes singly, lossy planes as
    # pairs once both are quantized; same order as quantize availability
    seen = set()
    steps = []  # (kind, kt)
    for kt in qorder:
        seen.add(kt)
        if kt < KTE:
            steps.append(("e", kt))
        else:
            mate = kt + 1 if kt % 2 == KTE % 2 else kt - 1
            if mate in seen:
                steps.append(("l", min(kt, mate)))

    with tile.TileContext(nc) as tc:
        with (
            tc.tile_pool(name="xin", bufs=4) as xin,  # [128,K] f32 x loads
            tc.tile_pool(name="wld", bufs=NCACHE) as wld,  # [128,K] f32 W
            tc.tile_pool(name="qtmp", bufs=2) as qtmp,  # qx bf16
            tc.tile_pool(name="qxt", bufs=4) as qxtp,  # [128,KT,128] bf16 x^T
            tc.tile_pool(name="qxl", bufs=4) as qxlp,  # [128,KTL,128] fp8
            tc.tile_pool(name="outp", bufs=2) as outp,  # [128,N] f16 results
            tc.tile_pool(name="scr", bufs=1) as scr,  # [128,K] bf16 |w| out
            tc.tile_pool(name="singles", bufs=1) as singles,
            tc.tile_pool(name="small", bufs=6) as small,  # [128,1] stats
            tc.tile_pool(name="pacc", bufs=8, space="PSUM") as pacc,
        ):
            ones_mat = singles.tile([128, 128], f32)
            nc.vector.memset(ones_mat, 1.0)
            qwT = singles.tile([128, KT, N], fp8)
            wsums = singles.tile([128, KT], f32)
            negmagic = singles.tile([128, 1], f32)
            nc.vector.memset(negmagic, -MAGIC)

            # ---- W stream on sync with x0..x2 loads slotted mid-stream
            # (late enough not to delay the mean, early enough for the ramp);
            # the scalar queue carries only ACT work so Abs starts immediately
            x_pre = {}

            def x_load(mi):
                x_t = xin.tile([128, K], f32, tag="xin", name=f"xl{mi}")
                nc.sync.dma_start(out=x_t, in_=x_ext[ts(mi, 128), :])
                x_pre[mi] = x_t

            wstream = {}
            for kt in range(KT):
                wt_t = wld.tile([128, K], f32, tag="wld", name=f"wld{kt}")
                nc.sync.dma_start(out=wt_t, in_=wt_ext[ts(kt, 128), :])
                wstream[kt] = wt_t
                if KT - 8 <= kt < KT - 8 + 2 * min(NPRE, MT) and (kt - KT + 8) % 2 == 0:
                    x_load((kt - KT + 8) // 2)
                aw = scr.tile([128, K], bf16, tag="scr")
                nc.scalar.activation(
                    out=aw, in_=wt_t, func=Act.Abs,
                    accum_out=wsums[:, kt : kt + 1],
                )
                for j in range(2):
                    pj = pacc.tile(
                        [128, 64], f32, tag="acc", name=f"jk{kt}_{j}"
                    )
                    nc.tensor.matmul(
                        pj, lhsT=wt_t[:, :128],
                        rhs=wt_t[:, 128 + 64 * j : 192 + 64 * j],
                        start=True, stop=True, skip_group_check=True,
                    )

            def x_quant(mi):
                if mi in x_pre:
                    x_t = x_pre[mi]
                else:
                    x_t = xin.tile([128, K], f32, tag="xin", name=f"x{mi}")
                    nc.sync.dma_start(out=x_t, in_=x_ext[ts(mi, 128), :])
                amax = small.tile([128, 1], f32, tag="small")
                nc.vector.tensor_reduce(
                    out=amax, in_=x_t, axis=AxX, op=Alu.max,
                    apply_absolute_value=True,
                )
                amc = small.tile([128, 1], f32, tag="amc", name=f"amc{mi}")
                nc.vector.tensor_scalar_max(out=amc, in0=amax, scalar1=1e-5)
                rec = small.tile([128, 1], f32, tag="small")
                nc.vector.reciprocal(out=rec, in_=amc)
                si = small.tile([128, 1], f32, tag="small")
                nc.vector.tensor_scalar_mul(out=si, in0=rec, scalar1=127.0)
                # u = x*si + MAGIC in place (DVE), qx = u - MAGIC -> bf16
                nc.vector.tensor_scalar(
                    out=x_t, in0=x_t, scalar1=si, scalar2=MAGIC,
                    op0=Alu.mult, op1=Alu.add,
                )
                qx = qtmp.tile([128, K], bf16, tag="qtmp")
                nc.vector.tensor_scalar_add(out=qx, in0=x_t, scalar1=-MAGIC)
                qxT = qxtp.tile(
                    [128, KT, 128], bf16, tag="qxt", name=f"qxT{mi}"
                )
                nc.sync.dma_start_transpose(out=qxT, in_=qx)
                # RNE fp8 copies of the lossy planes (paired for DoubleRow)
                qxl = qxlp.tile(
                    [128, KTL, 128], fp8, tag="qxl", name=f"qxl{mi}"
                )
                nc.vector.tensor_copy(out=qxl, in_=qxT[:, KTE:, :])
                return qxT, qxl, amc

            # x0/x1 chains first; the sw chain and the first rounds are
            # hoisted ahead of x2's DVE chain so Sign0 starts ~4us sooner
            xq = {}
            for mi in range(min(2, NPRE, MT)):
                xq[mi] = x_quant(mi)

            wtot = small.tile([128, 1], f32, tag="small")
            nc.vector.tensor_reduce(out=wtot, in_=wsums, axis=AxX, op=Alu.add)

            # ---- mean broadcast + scalar chain (before the re-read junk on
            # the PE queue: the re-reads wait on slot releases that need sw)
            ptot_b = pacc.tile([128, 1], f32, tag="acc", name="ptot_b")
            nc.tensor.matmul(ptot_b, lhsT=ones_mat, rhs=wtot, start=True, stop=True)

            # re-read of the first NRR tiles, right behind the stream.
            # No junk matmuls on them: they would head-of-line block the
            # ramp matmuls behind the slot-WAR'd re-read DMAs. The clock
            # bridge to the first real matmul is 2 tiny matmuls on the
            # freshly transposed qxT tiles instead.
            for kt in range(NRR):
                wt_t = wld.tile([128, K], f32, tag="wld", name=f"wldr{kt}")
                nc.sync.dma_start(out=wt_t, in_=wt_ext[ts(kt, 128), :])
                wstream[kt] = wt_t
            for mi in range(min(2, NPRE, MT)):
                qxT_j = xq[mi][0]
                pj = pacc.tile([128, 128], f32, tag="acc", name=f"jt{mi}")
                nc.tensor.matmul(
                    pj, lhsT=qxT_j[:, 0, :], rhs=qxT_j[:, 1, :],
                    start=True, stop=True, skip_group_check=True,
                )
            meanc_b = small.tile([128, 1], f32, tag="s1")
            nc.vector.tensor_scalar(
                out=meanc_b,
                in0=ptot_b,
                scalar1=1.0 / (K * N),
                scalar2=1e-5,
                op0=Alu.mult,
                op1=Alu.max,
            )
            sw_b = singles.tile([128, 1], f32)
            nc.vector.reciprocal(out=sw_b, in_=meanc_b)
            q_b = singles.tile([128, 1], f32)
            nc.vector.tensor_scalar_mul(out=q_b, in0=meanc_b, scalar1=1.0 / 127.0)

            # ---- W quantize, split across engines: DVE magic-round, then
            # ACT Sign(u-MAGIC)->fp8 for most planes; the last NDVEQ slots
            # run fully on DVE (sub+max, then min->fp8)
            def w_quant(qi, kt):
                wt_t = wstream[kt]
                nc.vector.tensor_scalar(
                    out=wt_t, in0=wt_t, scalar1=sw_b, scalar2=MAGIC,
                    op0=Alu.mult, op1=Alu.add,
                )
                if qi < KT - NDVEQ:
                    nc.scalar.activation(
                        out=qwT[:, kt, :], in_=wt_t, func=Act.Sign,
                        bias=negmagic,
                    )
                else:
                    nc.vector.tensor_scalar(
                        out=wt_t, in0=wt_t, scalar1=-MAGIC, scalar2=-1.0,
                        op0=Alu.add, op1=Alu.max,
                    )
                    nc.vector.tensor_scalar(
                        out=qwT[:, kt, :], in0=wt_t, scalar1=1.0, scalar2=1.0,
                        op0=Alu.mult, op1=Alu.min,
                    )

            qdone0 = 3 if MT >= 2 else 0
            for qi in range(qdone0):
                w_quant(qi, qorder[qi])
            for mi in range(2, min(NPRE, MT)):
                xq[mi] = x_quant(mi)

            # ---- matmuls: bf16 x fp8 for exact planes; fp8 DoubleRow
            # pairs (2 k-planes per instr) for the lossy planes
            def mm(acc, qxT, qxl, step, nq, start, stop):
                kind, kt = step
                if kind == "e":
                    nc.tensor.matmul(
                        acc, lhsT=qxT[:, kt, :], rhs=qwT[:, kt, ts(nq, 512)],
                        start=start, stop=stop, skip_group_check=True,
                    )
                else:
                    nc.tensor.matmul(
                        acc,
                        lhsT=qxl[:, kt - KTE : kt - KTE + 2, :],
                        rhs=qwT[:, kt : kt + 2, ts(nq, 512)],
                        start=start, stop=stop,
                        perf_mode=DR, skip_group_check=True,
                    )

            def finish(mi, accs, amc, split=False):
                cs = small.tile([128, 1], f32, tag="small")
                nc.vector.tensor_mul(cs, amc, q_b)  # (amax/127)*meanc
                o_t = outp.tile([128, N], f16, tag="outp", name=f"o{mi}")
                for nq in range(NQ):
                    nc.scalar.activation(
                        out=o_t[:, ts(nq, 512)], in_=accs[nq],
                        func=Act.Copy, scale=cs,
                    )
                    if split:
                        nc.scalar.dma_start(
                            out=out_ext[ts(mi, 128), ts(nq, 512)],
                            in_=o_t[:, ts(nq, 512)],
                        )
                if not split:
                    nc.scalar.dma_start(out=out_ext[ts(mi, 128), :], in_=o_t)

            NS = len(steps)
            if MT >= 2:
                # ramp: row tiles 0,1 interleaved across the quantize burst
                qxT0, qxl0, amc0 = xq[0]
                qxT1, qxl1, amc1 = xq[1]
                accs0 = [
                    pacc.tile([128, 512], f32, tag="acc", name=f"acc_0_{i}")
                    for i in range(NQ)
                ]
                accs1 = [
                    pacc.tile([128, 512], f32, tag="acc", name=f"acc_1_{i}")
                    for i in range(NQ)
                ]
                qdone = qdone0
                for si_, step in enumerate(steps):
                    # emit quantizes needed for this step
                    need = qorder.index(step[1]) + 1 if step[0] == "e" else max(
                        qorder.index(step[1]), qorder.index(step[1] + 1)
                    ) + 1
                    while qdone < need:
                        w_quant(qdone, qorder[qdone])
                        qdone += 1
                    st, sp = si_ == 0, si_ == NS - 1
                    for nq in range(NQ):
                        mm(accs0[nq], qxT0, qxl0, step, nq, st, sp)
                    for nq in range(NQ):
                        mm(accs1[nq], qxT1, qxl1, step, nq, st, sp)
                while qdone < KT:
                    w_quant(qdone, qorder[qdone])
                    qdone += 1
                finish(0, accs0, amc0)
                finish(1, accs1, amc1)
                start_mi = 2
                for la in range(NPRE, min(NPRE + 2, MT)):
                    x_load(la)
            else:
                for qi, kt in enumerate(qorder):
                    w_quant(qi, kt)
                start_mi = 0

            for mi in range(start_mi, MT):
                qxT, qxl, amc = xq[mi] if mi in xq else x_quant(mi)
                if mi + 2 + NPRE - start_mi < MT and mi >= NPRE - 1:
                    x_load(mi + 2 + NPRE - start_mi)
                accs = [
                    pacc.tile([128, 512], f32, tag="acc", name=f"acc_{mi}_{i}")
                    for i in range(NQ)
                ]
                if mi == MT - 1:
                    # nq-inner: shorter kernel tail
                    for nq in range(NQ):
                        for si_, step in enumerate(steps):
                            mm(accs[nq], qxT, qxl, step, nq,
                               si_ == 0, si_ == NS - 1)
                else:
                    for si_, step in enumerate(steps):
                        st, sp = si_ == 0, si_ == NS - 1
                        for nq in range(NQ):
                            mm(accs[nq], qxT, qxl, step, nq, st, sp)
                finish(mi, accs, amc, split=(mi == MT - 1))

    nc.compile()
    return nc


_NC_CACHE = {}


def _get_nc(rows_per_core):
    if rows_per_core not in _NC_CACHE:
        _NC_CACHE[rows_per_core] = build(rows_per_core)
    return _NC_CACHE[rows_per_core]


def run(x, weight, **spmd_kwargs):
    x = np.ascontiguousarray(np.asarray(x, dtype=np.float32))
    weight = np.asarray(weight, dtype=np.float32)
    b, s, k = x.shape
    rows = b * s
    rpc = rows // N_CORES
    xr = x.reshape(rows, k)
    wt = np.ascontiguousarray(weight.T)
    nc = _get_nc(rpc)
    in_maps = [
        {"x": xr[i * rpc : (i + 1) * rpc], "wt": wt} for i in range(N_CORES)
    ]
    res = run_bass_kernel_spmd(
        nc, in_maps, core_ids=list(range(N_CORES)), **spmd_kwargs
    )
    out = np.concatenate(
        [res.results[i]["out"] for i in range(N_CORES)], axis=0
    )
    return out.reshape(b, s, N), res


def kernel(x, weight):
    out, _ = run(x, weight)
    return out
